# revision 10
# baseline (speedup 1.0000x reference)
"""Trainium2 Bass kernel for Transformer-XL relative attention (nn_Attention).

Sharding: 8 cores = data-parallel over batch (2) x tensor-parallel over heads
(16 -> 4 per core).  Each core computes its 4 heads' attention for its batch,
a partial output projection, then ReduceScatter(add) over its batch quad;
each core LayerNorms its 512-row output shard.

Device-side structure (per core):
- fp32r matmuls (full PE rate at free-dim >= 256).
- The reference's _rel_shift (shear with cross-row wraparound) is computed
  exactly via a flat DRAM buffer: bd rows written at stride L+1 with a
  leading zero; rows of length L re-read from offset L give the shifted
  matrix.  Contiguous bf16 DMA both directions.
- scores(nat) = ac matmul (K=64) + identity-matmul add of shifted bd (bf16).
- scores -> bf16 -> PE transpose-mode -> bf16 PSUM -> exp on ACT gives P^T
  in SBUF for the context matmul.
- mask + softmax denominator ride the context matmul: V+ = [V*mask | mask],
  so psum row 64 is the masked denominator (masked cols contribute exact 0,
  = reference's exp(-inf)).
"""

import numpy as np

B, L, D, NH, DH = 2, 2048, 1024, 16, 64
NHL = 4
P = 128
SCALE = 1.0 / np.sqrt(DH)
LN_EPS = 1e-5
N_CORES = 8

_CACHE = {}


def _build_program():
    import concourse.bacc as bacc
    import concourse.mybir as mybir
    import concourse.tile as tile
    from concourse.masks import make_identity

    F32 = mybir.dt.float32
    F16 = mybir.dt.float16
    AF = mybir.ActivationFunctionType
    AX = mybir.AxisListType
    OP = mybir.AluOpType

    nc = bacc.Bacc("TRN2", target_bir_lowering=False, debug=False,
                   num_devices=N_CORES)

    xT = nc.declare_dram_parameter("xT", [D, L], F32, isOutput=False)
    relT = nc.declare_dram_parameter("relT", [D, L], F32, isOutput=False)
    xres = nc.declare_dram_parameter("xres", [512, D], F32, isOutput=False)
    Wq = nc.declare_dram_parameter("Wq", [D, 256], F32, isOutput=False)
    Wk = nc.declare_dram_parameter("Wk", [D, 256], F32, isOutput=False)
    Wv = nc.declare_dram_parameter("Wv", [D, 256], F32, isOutput=False)
    Wrel = nc.declare_dram_parameter("Wrel", [D, 256], F32, isOutput=False)
    Wout = nc.declare_dram_parameter("Wout", [256, D], F32, isOutput=False)
    rwb = nc.declare_dram_parameter("rwb", [256], F32, isOutput=False)
    rrb = nc.declare_dram_parameter("rrb", [256], F32, isOutput=False)
    mask01 = nc.declare_dram_parameter("mask01", [L], F32, isOutput=False)
    gamma = nc.declare_dram_parameter("gamma", [D], F32, isOutput=False)
    beta = nc.declare_dram_parameter("beta", [D], F32, isOutput=False)
    out = nc.declare_dram_parameter("out", [512, D], F32, isOutput=True)

    with tile.TileContext(nc) as tc:
        with (
            tc.tile_pool(name="persist", bufs=1) as pers,
            tc.tile_pool(name="dram", bufs=1, space="DRAM") as dram,
        ):
            ident = pers.tile([P, P], F16)
            make_identity(nc, ident[:])
            ones_r = pers.tile([1, 64], F16)
            nc.vector.memset(ones_r[:], 1.0)
            nbias = pers.tile([P, 1], F32)
            nc.vector.memset(nbias[:], -4.0)
            m01 = pers.tile([P, 16], F32)
            nc.sync.dma_start(m01[:], mask01.rearrange("(o p) -> p o", p=P))

            rwT = [pers.tile([P, L], F16, name=f"rwT{c}") for c in range(2)]
            rrT = [pers.tile([P, L], F16, name=f"rrT{c}") for c in range(2)]
            kT = [pers.tile([P, L], F16, name=f"kT{c}") for c in range(2)]
            rkT = [pers.tile([P, L], F16, name=f"rkT{c}") for c in range(2)]
            vp = [pers.tile([P, 16, DH + 1], F16, name=f"vp{h}") for h in range(NHL)]
            ctxT = [pers.tile([P, L], F16, name=f"ctxT{c}") for c in range(2)]

            # ---------- Phase A: projections ----------
            with (
                tc.tile_pool(name="slab", bufs=2) as slab_p,
                tc.tile_pool(name="s32", bufs=1) as s32_p,
                tc.tile_pool(name="wr", bufs=1) as wr_p,
                tc.tile_pool(name="psA", bufs=2, space="PSUM") as psA,
            ):
                wq_r = wr_p.tile([P, 8, 256], F16)
                wk_r = wr_p.tile([P, 8, 256], F16)
                wv_r = wr_p.tile([P, 8, 256], F16)
                wl_r = wr_p.tile([P, 8, 256], F16)
                for wdram, wr in ((Wq, wq_r), (Wk, wk_r), (Wv, wv_r), (Wrel, wl_r)):
                    wt32 = s32_p.tile([P, 8, 256], F32, tag="w32")
                    nc.sync.dma_start(wt32[:], wdram.rearrange("(k p) n -> p k n", p=P))
                    nc.vector.tensor_copy(wr[:], wt32[:])

                rwb_sb = wr_p.tile([P, 2], F32)
                nc.sync.dma_start(rwb_sb[:], rwb.rearrange("(c p) -> p c", p=P))
                rrb_sb = wr_p.tile([P, 2], F32)
                nc.sync.dma_start(rrb_sb[:], rrb.rearrange("(c p) -> p c", p=P))

                for ic in range(8):
                    I0 = 256 * ic
                    xs32 = s32_p.tile([P, 8, 256], F32, tag="s32x")
                    nc.sync.dma_start(
                        xs32[:], xT[:, I0:I0 + 256].rearrange("(k p) n -> p k n", p=P))
                    xs = slab_p.tile([P, 8, 256], F16, tag="xs")
                    nc.vector.tensor_copy(xs[:], xs32[:])
                    rs32 = s32_p.tile([P, 8, 256], F32, tag="s32r")
                    nc.sync.dma_start(
                        rs32[:], relT[:, I0:I0 + 256].rearrange("(k p) n -> p k n", p=P))
                    rsl = slab_p.tile([P, 8, 256], F16, tag="rsl")
                    nc.vector.tensor_copy(rsl[:], rs32[:])

                    for cc in range(2):
                        pq = psA.tile([P, 256], F32, tag="pA")
                        for k in range(8):
                            nc.tensor.matmul(pq[:], wq_r[:, k, 128 * cc:128 * cc + 128],
                                             xs[:, k, :], start=(k == 0), stop=(k == 7))
                        nc.vector.tensor_scalar_add(rwT[cc][:, I0:I0 + 256], pq[:],
                                                    rwb_sb[:, cc:cc + 1])
                        nc.vector.tensor_scalar_add(rrT[cc][:, I0:I0 + 256], pq[:],
                                                    rrb_sb[:, cc:cc + 1])
                        pk = psA.tile([P, 256], F32, tag="pA")
                        for k in range(8):
                            nc.tensor.matmul(pk[:], wk_r[:, k, 128 * cc:128 * cc + 128],
                                             xs[:, k, :], start=(k == 0), stop=(k == 7))
                        nc.vector.tensor_copy(kT[cc][:, I0:I0 + 256], pk[:])
                        pr = psA.tile([P, 256], F32, tag="pA")
                        for k in range(8):
                            nc.tensor.matmul(pr[:], wl_r[:, k, 128 * cc:128 * cc + 128],
                                             rsl[:, k, :], start=(k == 0), stop=(k == 7))
                        nc.vector.tensor_copy(rkT[cc][:, I0:I0 + 256], pr[:])

                    for jj in range(2):
                        jo = 2 * ic + jj
                        pv = psA.tile([P, 256], F32, tag="pA")
                        for k in range(8):
                            nc.tensor.matmul(pv[:], xs[:, k, 128 * jj:128 * jj + 128],
                                             wv_r[:, k, :], start=(k == 0), stop=(k == 7))
                        for h in range(NHL):
                            nc.vector.tensor_scalar_mul(
                                vp[h][:, jo, 0:DH], pv[:, DH * h:DH * h + DH],
                                m01[:, jo:jo + 1])
                            nc.vector.tensor_copy(vp[h][:, jo, DH:DH + 1],
                                                  m01[:, jo:jo + 1])

            # ---------- Phase B: attention ----------
            pf_bufs = [dram.tile([L * (L + 1)], F16, name=f"pf{i}") for i in range(2)]

            with (
                tc.tile_pool(name="wt", bufs=3) as wt_p,
                tc.tile_pool(name="sh", bufs=3) as sh_p,
                tc.tile_pool(name="s16", bufs=9) as s16_p,
                tc.tile_pool(name="pt", bufs=3) as pt_p,
                tc.tile_pool(name="bc", bufs=2) as bc_p,
                tc.tile_pool(name="oddt", bufs=2) as odd_p,
                tc.tile_pool(name="den", bufs=4) as den_p,
                tc.tile_pool(name="psB", bufs=1, space="PSUM") as psB,
                tc.tile_pool(name="psS", bufs=3, space="PSUM") as psS,
                tc.tile_pool(name="psT", bufs=2, space="PSUM") as psT,
                tc.tile_pool(name="psBc", bufs=2, space="PSUM") as psBc,
            ):

                for h in range(NHL):
                    pf = pf_bufs[h % 2][:]
                    cc, par = h // 2, h % 2
                    sA = slice(64 * par, 64 * par + 64)
                    pf2d = pf[0:L * (L + 1)].rearrange("(r c) -> r c", c=L + 1)

                    for ic in range(16):
                        I0 = 128 * ic
                        wt = wt_p.tile([P, 2049], F16, tag="wt")
                        nc.vector.memset(wt[:, 0:1], 0.0)
                        for t in range(4):
                            pbd = psB.tile([P, 512], F32, tag="bd")
                            nc.tensor.matmul(pbd[:],
                                             rrT[cc][sA, I0:I0 + 128],
                                             rkT[cc][sA, 512 * t:512 * t + 512],
                                             start=True, stop=True)
                            nc.vector.tensor_copy(
                                wt[:, 1 + 512 * t: 1 + 512 * t + 512], pbd[:])
                        nc.gpsimd.dma_start(pf2d[I0:I0 + 128, :], wt[:])

                    ot = odd_p.tile([64, L], F16, tag="odd", name="ot") if par == 1 else None

                    for half in range(2):
                        H0 = 1024 * half
                        s16s = []
                        for icc in range(8):
                            I0 = H0 + 128 * icc
                            sh16 = sh_p.tile([P, L], F16, tag="sh")
                            nc.gpsimd.dma_start(
                                sh16[:],
                                pf[L + I0 * L: L + (I0 + 128) * L]
                                .rearrange("(r c) -> r c", c=L))
                            s16 = s16_p.tile([P, L], F16, tag="s16")
                            for t in range(4):
                                psc = psS.tile([P, 512], F32, tag="sc")
                                nc.tensor.matmul(psc[:],
                                                 rwT[cc][sA, I0:I0 + 128],
                                                 kT[cc][sA, 512 * t:512 * t + 512],
                                                 start=True, stop=False)
                                nc.tensor.matmul(psc[:], ident[:],
                                                 sh16[:, 512 * t:512 * t + 512],
                                                 start=False, stop=True)
                                if t % 2 == 1:
                                    nc.vector.tensor_copy(
                                        s16[:, 512 * t:512 * t + 512], psc[:])
                                else:
                                    nc.scalar.copy(
                                        s16[:, 512 * t:512 * t + 512], psc[:])
                            s16s.append(s16)

                        pc0 = psBc.tile([65, 512], F32, tag="c")
                        pc1 = psBc.tile([65, 512], F32, tag="c")
                        for J in range(16):
                            ptp = psT.tile([P, 1024], F16, tag="pt")
                            for icc in range(8):
                                nc.tensor.matmul(ptp[:, 128 * icc:128 * icc + 128],
                                                 s16s[icc][:, 128 * J:128 * J + 128],
                                                 ident[:], is_transpose=True,
                                                 start=True, stop=True)
                            pt_sb = pt_p.tile([P, 1024], F16, tag="ptsb")
                            nc.scalar.activation(pt_sb[:], ptp[:], AF.Exp, bias=nbias[:])
                            for ii, pc in enumerate((pc0, pc1)):
                                nc.tensor.matmul(pc[:],
                                                 vp[h][:, J, :],
                                                 pt_sb[:, 512 * ii:512 * ii + 512],
                                                 start=(J == 0), stop=(J == 15))

                        den_sb = den_p.tile([P, 1024], F32, tag="den_sb", name="den_sb")
                        den0 = den_p.tile([1, 1024], F32, tag="den0", name="den0")
                        rec0 = den_p.tile([1, 1024], F32, tag="rec0", name="rec0")
                        recr = den_p.tile([1, 1024], F16, tag="recr", name="recr")
                        scr = den_p.tile([1, 1024], F32, tag="scr", name="scr")
                        nc.vector.tensor_copy(den_sb[64:65, 0:512], pc0[64:65, :])
                        nc.vector.tensor_copy(den_sb[64:65, 512:1024], pc1[64:65, :])
                        nc.sync.dma_start(den0[0:1, :], den_sb[64:65, 0:1024])
                        nc.vector.reciprocal_approx_accurate(
                            rec0[0:1, :], den0[0:1, :], scr[0:1, :])
                        nc.vector.tensor_copy(recr[0:1, :], rec0[0:1, :])
                        for ii, pc in enumerate((pc0, pc1)):
                            i0 = H0 + 512 * ii
                            pb = psS.tile([P, 512], F32, tag="sc")
                            nc.tensor.matmul(pb[0:64, :], ones_r[0:1, :],
                                             recr[0:1, 512 * ii:512 * ii + 512],
                                             start=True, stop=True)
                            bc = bc_p.tile([64, 512], F32, tag="bc")
                            nc.vector.tensor_copy(bc[:], pb[0:64, :])
                            if par == 0:
                                nc.vector.tensor_mul(ctxT[cc][0:64, i0:i0 + 512],
                                                     pc[0:64, :], bc[:])
                            else:
                                nc.vector.tensor_mul(ot[:, i0:i0 + 512],
                                                     pc[0:64, :], bc[:])
                    if par == 1:
                        nc.sync.dma_start(ctxT[cc][64:128, :], ot[:, :])

            # ---------- Phase C: out projection + ReduceScatter + LayerNorm ----
            attn_d = dram.tile([L, D], F32)
            rs_d = dram.tile([512, D], F32)

            with (
                tc.tile_pool(name="wo", bufs=1) as wo_p,
                tc.tile_pool(name="oc", bufs=3) as oc_p,
                tc.tile_pool(name="psC", bufs=2, space="PSUM") as psC,
            ):
                wo_r = [wo_p.tile([P, 2, 512], F16, name=f"wo{c}") for c in range(2)]
                for c in range(2):
                    w32 = oc_p.tile([P, 2, 512], F32, tag="w32c")
                    nc.sync.dma_start(
                        w32[:], Wout[128 * c:128 * c + 128, :]
                        .rearrange("p (t n) -> p t n", t=2))
                    nc.vector.tensor_copy(wo_r[c][:], w32[:])

                for ic in range(16):
                    I0 = 128 * ic
                    for t in range(2):
                        po = psC.tile([P, 512], F32, tag="po")
                        for c in range(2):
                            nc.tensor.matmul(po[:], ctxT[c][:, I0:I0 + 128],
                                             wo_r[c][:, t, :],
                                             start=(c == 0), stop=(c == 1))
                        ao = oc_p.tile([P, 512], F32, tag="ao")
                        nc.vector.tensor_copy(ao[:], po[:])
                        nc.sync.dma_start(
                            attn_d[I0:I0 + 128, 512 * t:512 * t + 512], ao[:])

                nc.gpsimd.collective_compute(
                    "ReduceScatter", OP.add,
                    replica_groups=[[0, 1, 2, 3], [4, 5, 6, 7]],
                    ins=[attn_d[:].opt()], outs=[rs_d[:].opt()],
                )

            with (
                tc.tile_pool(name="ln", bufs=2) as ln_p,
                tc.tile_pool(name="lng", bufs=1) as lng_p,
            ):
                gb = lng_p.tile([P, D], F32)
                nc.gpsimd.dma_start(gb[:], gamma.ap().rearrange("(o d) -> o d", o=1).to_broadcast((P, D)))
                bb = lng_p.tile([P, D], F32)
                nc.gpsimd.dma_start(bb[:], beta.ap().rearrange("(o d) -> o d", o=1).to_broadcast((P, D)))

                for rc in range(4):
                    R0 = 128 * rc
                    zt = ln_p.tile([P, D], F32, tag="zt")
                    nc.sync.dma_start(zt[:], rs_d[R0:R0 + 128, :])
                    xr = ln_p.tile([P, D], F32, tag="xr")
                    nc.sync.dma_start(xr[:], xres[R0:R0 + 128, :])
                    nc.vector.tensor_add(zt[:], zt[:], xr[:])
                    mu = ln_p.tile([P, 1], F32, tag="mu")
                    nc.vector.tensor_reduce(mu[:], zt[:], AX.X, OP.add)
                    nc.vector.tensor_scalar_mul(mu[:], mu[:], 1.0 / D)
                    xc = ln_p.tile([P, D], F32, tag="xc")
                    nc.vector.tensor_scalar_sub(xc[:], zt[:], mu[:])
                    sq = ln_p.tile([P, D], F32, tag="sq")
                    nc.vector.tensor_mul(sq[:], xc[:], xc[:])
                    var = ln_p.tile([P, 1], F32, tag="var")
                    nc.vector.tensor_reduce(var[:], sq[:], AX.X, OP.add)
                    nc.vector.tensor_scalar_mul(var[:], var[:], 1.0 / D)
                    nc.vector.tensor_scalar_add(var[:], var[:], LN_EPS)
                    sd = ln_p.tile([P, 1], F32, tag="sd")
                    nc.scalar.activation(sd[:], var[:], AF.Sqrt)
                    isd = ln_p.tile([P, 1], F32, tag="isd")
                    nc.vector.reciprocal(isd[:], sd[:])
                    nc.vector.tensor_scalar_mul(xc[:], xc[:], isd[:])
                    nc.vector.tensor_mul(xc[:], xc[:], gb[:])
                    nc.vector.tensor_add(xc[:], xc[:], bb[:])
                    nc.sync.dma_start(out[R0:R0 + 128, :], xc[:])

    nc.compile()
    return nc


def _prep_inputs(x, relative_pos, r_w_bias, r_r_bias, attn_mask,
                 W_qkv, W_rel, W_out, ln_gamma, ln_beta):
    in_maps = []
    relT = np.ascontiguousarray(relative_pos.T).astype(np.float32)
    m01f = (~np.asarray(attn_mask).astype(bool)).astype(np.float32)
    for c in range(N_CORES):
        b, g = c // 4, c % 4
        h0 = 4 * g
        cols = slice(DH * h0, DH * h0 + 256)
        im = dict(
            xT=np.ascontiguousarray(x[b].T).astype(np.float32),
            relT=relT,
            xres=np.ascontiguousarray(
                x[b, 512 * g:512 * g + 512, :]).astype(np.float32),
            Wq=np.ascontiguousarray(
                W_qkv[:, DH * h0:DH * h0 + 256] * SCALE).astype(np.float32),
            Wk=np.ascontiguousarray(
                W_qkv[:, D + DH * h0: D + DH * h0 + 256]).astype(np.float32),
            Wv=np.ascontiguousarray(
                W_qkv[:, 2 * D + DH * h0: 2 * D + DH * h0 + 256]).astype(np.float32),
            Wrel=np.ascontiguousarray(W_rel[:, cols]).astype(np.float32),
            Wout=np.ascontiguousarray(W_out[cols, :]).astype(np.float32),
            rwb=np.ascontiguousarray(
                r_w_bias[h0:h0 + 4].reshape(-1) * SCALE).astype(np.float32),
            rrb=np.ascontiguousarray(
                r_r_bias[h0:h0 + 4].reshape(-1) * SCALE).astype(np.float32),
            mask01=m01f[b],
            gamma=np.asarray(ln_gamma).astype(np.float32),
            beta=np.asarray(ln_beta).astype(np.float32),
        )
        in_maps.append(im)
    return in_maps


def kernel(**inputs):
    from concourse.bass_utils import run_bass_kernel_spmd

    if "nc" not in _CACHE:
        _CACHE["nc"] = _build_program()
    nc = _CACHE["nc"]

    in_maps = _prep_inputs(**{k: np.asarray(v) for k, v in inputs.items()})
    res = run_bass_kernel_spmd(nc, in_maps, list(range(N_CORES)))
    outp = np.empty((B, L, D), np.float32)
    for c in range(N_CORES):
        b, g = c // 4, c % 4
        outp[b, 512 * g:512 * g + 512, :] = res.results[c]["out"]
    return outp
